# revision 5
# baseline (speedup 1.0000x reference)
"""GCN (7-layer, PyG GCNConv-style) on 8 Trainium2 NeuronCores.

Strategy (graph-partition data parallel, per sharding hint):
- Nodes are destination-sharded contiguously: core k owns nodes
  [k*12500, (k+1)*12500). Each core aggregates messages for its own nodes.
- Per layer: each core computes h~ = dinv * (H @ W) for its local nodes
  (bf16), AllGathers the full node-feature table to DRAM, then gathers
  source rows for 128-edge chunks with BATCHED native indirect DMAs
  (K chunks per DMA instruction to amortize SWDGE descriptor-generation
  overhead), and scatter-adds into PSUM via a selection-matrix matmul on
  the TensorEngine. The selection matrix folds the edge weight in:
  S[e, d] = w_e * (dst_pos(e) == d), built in ONE fused DVE
  tensor_scalar op (is_equal then mult, 4x perf mode on bf16).
- Self-loops fold in algebraically: out = dinv*acc + dinv^2*(H@W) + b;
  the dinv^2 term (htil2) is produced by a second scaled PSUM->SBUF copy
  in stage A on the Activation engine.
- Degrees (1 + sum of incoming edge weights) are computed on device by row
  reduction over a host-permuted, zero-padded copy of edge_weight;
  dinv = rsqrt(deg) on the scalar engine.

Host-side work is index/layout preparation only (sharding, edge sorting,
padding, dtype casts); all floating-point math runs on device.
"""
import sys

sys.path.insert(0, "/opt/trn_rl_repo")

from contextlib import ExitStack

import numpy as np
import ml_dtypes

BF16 = ml_dtypes.bfloat16

NC = 8
N_NODES = 100000
NLOC = N_NODES // NC            # 12500
NBLK = (NLOC + 127) // 128      # 98
NLOCP = NBLK * 128              # 12544 (padded local nodes)
NTAB = NC * NLOCP               # padded global table rows
DIMS = [(128, 50), (50, 50), (50, 30), (30, 30), (30, 10), (10, 10), (10, 1)]
NLAYER = len(DIMS)
KGRP = 128                      # edge chunks per batched indirect gather


def _host_prep(x, edge_index, edge_weight):
    """Shard + sort edges, build per-core device input arrays."""
    row = np.asarray(edge_index[0], dtype=np.int64)
    col = np.asarray(edge_index[1], dtype=np.int64)
    w = np.asarray(edge_weight, dtype=np.float32)

    core_of = col // NLOC
    per_core = []
    blk_cnt_max = np.zeros(NBLK, np.int64)
    max_deg = 1
    for k in range(NC):
        m = core_of == k
        r_k = row[m]
        c_k = col[m] - k * NLOC
        w_k = w[m]
        blk = c_k // 128
        pos = c_k % 128
        order = np.argsort(blk, kind="stable")
        r_k, c_k, w_k, blk, pos = (a[order] for a in (r_k, c_k, w_k, blk, pos))
        cnt = np.bincount(blk, minlength=NBLK)
        blk_cnt_max = np.maximum(blk_cnt_max, cnt)
        # in-degree count for wpad sizing
        cdeg = np.bincount(c_k, minlength=NLOCP)
        max_deg = max(max_deg, int(cdeg.max()))
        per_core.append((r_k, c_k, w_k, blk, pos, cnt, cdeg))

    M_b = np.maximum(1, np.ceil(blk_cnt_max / 128).astype(np.int64))
    cum = np.zeros(NBLK + 1, np.int64)
    cum[1:] = np.cumsum(M_b)
    NCH = int(cum[-1])
    D = max_deg

    in_maps = []
    for k in range(NC):
        r_k, c_k, w_k, blk, pos, cnt, cdeg = per_core[k]
        n_e = len(r_k)
        # slot within the destination block's edge run
        first = np.zeros(NBLK + 1, dtype=np.int64)
        first[1:] = np.cumsum(cnt)
        rank = np.arange(n_e, dtype=np.int64) - first[blk]
        chunk = cum[blk] + rank // 128
        part = rank % 128

        g_idx = np.zeros((128, NCH), np.int32)
        dst_pos = np.zeros((128, NCH), np.float32)
        w_e = np.zeros((128, NCH), np.float32)
        # padded global table id of the source node (row-major over the
        # allgathered table [(core*128+p)*NBLK + c], matching node
        # (core, c*128+p) stored at table[core*128+p, c*fo:(c+1)*fo])
        loc = r_k % NLOC
        src_pad = (r_k // NLOC) * NLOCP + (loc % 128) * NBLK + loc // 128
        g_idx[part, chunk] = src_pad.astype(np.int32)
        dst_pos[part, chunk] = pos.astype(np.float32)
        w_e[part, chunk] = w_k

        # padded per-node incoming weights for degree computation
        order2 = np.argsort(c_k, kind="stable")
        c_s = c_k[order2]
        w_s = w_k[order2]
        nfirst = np.zeros(NLOCP + 1, np.int64)
        nfirst[1:] = np.cumsum(np.bincount(c_s, minlength=NLOCP))
        nrank = np.arange(len(c_s), dtype=np.int64) - nfirst[c_s]
        wpad = np.zeros((NLOCP, D), np.float32)
        wpad[c_s, nrank] = w_s
        # device layout [128, NBLK, D]: node c*128+p -> [p, c, :]
        wpad_dev = wpad.reshape(NBLK, 128, D).transpose(1, 0, 2).copy()

        xk = np.zeros((NLOCP, x.shape[1]), np.float32)
        xk[:NLOC] = x[k * NLOC : (k + 1) * NLOC]
        # device layout [128, NBLK, F]: node c*128+p -> [p, c, :]
        xk_dev = np.ascontiguousarray(
            xk.reshape(NBLK, 128, x.shape[1]).transpose(1, 0, 2)
        ).astype(BF16)

        in_maps.append(
            {
                "x_p": xk_dev,
                "g_idx": g_idx,
                "dst_pos": dst_pos,
                "w_e": w_e,
                "wpad": wpad_dev,
            }
        )
    return in_maps, M_b, cum, D


def _build_program(M_b, cum, D, weights_shapes):
    from concourse import bass, bacc, mybir, tile

    NCH = int(cum[-1])
    nc = bacc.Bacc("TRN2", target_bir_lowering=False, debug=False, num_devices=NC)

    f32 = mybir.dt.float32
    bf16 = mybir.dt.bfloat16

    x_p = nc.declare_dram_parameter("x_p", [128, NBLK, 128], bf16, isOutput=False)
    g_idx = nc.declare_dram_parameter("g_idx", [128, NCH], mybir.dt.int32, isOutput=False)
    dst_pos = nc.declare_dram_parameter("dst_pos", [128, NCH], f32, isOutput=False)
    w_e = nc.declare_dram_parameter("w_e", [128, NCH], f32, isOutput=False)
    wpad = nc.declare_dram_parameter("wpad", [128, NBLK, D], f32, isOutput=False)
    iota = nc.declare_dram_parameter("iota", [128, 128], bf16, isOutput=False)
    ident = nc.declare_dram_parameter("ident", [128, 128], bf16, isOutput=False)
    Ws, Bs = [], []
    for i, (fi, fo) in enumerate(DIMS):
        Ws.append(nc.declare_dram_parameter(f"W{i+1}", [fi, fo], bf16, isOutput=False))
        bdt = f32 if i == NLAYER - 1 else bf16
        Bs.append(nc.declare_dram_parameter(f"b{i+1}", [128, fo], bdt, isOutput=False))
    out_ext = nc.declare_dram_parameter("out", [128, NBLK], f32, isOutput=True)

    bounces = [nc.dram_tensor(f"bounce{i}", [128, NBLK * fo], bf16) for i, (fi, fo) in enumerate(DIMS)]
    tables = [
        nc.dram_tensor(f"table{i}", [NC * 128, NBLK * fo], bf16, addr_space="Shared")
        for i, (fi, fo) in enumerate(DIMS)
    ]

    with tile.TileContext(nc) as tc, ExitStack() as ctx:
        const = ctx.enter_context(tc.tile_pool(name="const", bufs=1))
        work = ctx.enter_context(tc.tile_pool(name="work", bufs=2))
        hpool = ctx.enter_context(tc.tile_pool(name="hpool", bufs=2))
        tpool = ctx.enter_context(tc.tile_pool(name="tpool", bufs=2))
        msgp = ctx.enter_context(tc.tile_pool(name="msgp", bufs=4))
        selp = ctx.enter_context(tc.tile_pool(name="selp", bufs=12))
        psT = ctx.enter_context(tc.tile_pool(name="psT", bufs=2, space="PSUM"))
        psH = ctx.enter_context(tc.tile_pool(name="psH", bufs=2, space="PSUM"))
        psA = ctx.enter_context(tc.tile_pool(name="psA", bufs=4, space="PSUM"))

        iota_t = const.tile([128, 128], bf16)
        nc.sync.dma_start(out=iota_t[:], in_=iota[:])
        ident_t = const.tile([128, 128], bf16)
        nc.sync.dma_start(out=ident_t[:], in_=ident[:])
        idx_t = const.tile([128, NCH], mybir.dt.int32)
        nc.sync.dma_start(out=idx_t[:], in_=g_idx[:])
        pos_t = const.tile([128, NCH], f32)
        nc.sync.dma_start(out=pos_t[:], in_=dst_pos[:])
        wgt_t = const.tile([128, NCH], f32)
        nc.sync.dma_start(out=wgt_t[:], in_=w_e[:])
        x_sb = const.tile([128, NBLK, 128], bf16)
        nc.sync.dma_start(out=x_sb[:], in_=x_p[:])
        W_ts, B_ts = [], []
        for i, (fi, fo) in enumerate(DIMS):
            W_t = const.tile([fi, fo], bf16, tag=f"W{i}")
            nc.sync.dma_start(out=W_t[:], in_=Ws[i][:])
            bdt = f32 if i == NLAYER - 1 else bf16
            B_t = const.tile([128, fo], bdt, tag=f"B{i}")
            nc.sync.dma_start(out=B_t[:], in_=Bs[i][:])
            W_ts.append(W_t)
            B_ts.append(B_t)

        # ---- degree -> dinv, dinv^2 ----
        with tc.tile_pool(name="wpool", bufs=1) as wpool:
            wpad_t = wpool.tile([128, NBLK, D], f32, tag="wpad")
            nc.sync.dma_start(out=wpad_t[:], in_=wpad[:])
            deg_t = const.tile([128, NBLK], f32)
            for c in range(NBLK):
                nc.vector.tensor_reduce(
                    deg_t[:, c : c + 1],
                    wpad_t[:, c, :],
                    mybir.AxisListType.X,
                    mybir.AluOpType.add,
                )
            sqrt_t = const.tile([128, NBLK], f32)
            # dinv = 1 / sqrt(deg + 1)  (+1 = self-loop weight)
            nc.scalar.activation(
                out=sqrt_t[:], in_=deg_t[:], func=mybir.ActivationFunctionType.Sqrt, bias=1.0, scale=1.0
            )
            dinv_t = const.tile([128, NBLK], f32)
            nc.vector.reciprocal(out=dinv_t[:], in_=sqrt_t[:])
            dinv2_t = const.tile([128, NBLK], f32)
            nc.vector.tensor_tensor(
                out=dinv2_t[:], in0=dinv_t[:], in1=dinv_t[:], op=mybir.AluOpType.mult
            )

        h_cur = None  # SBUF tile [128, NBLK, F_in] bf16 for layers >= 2
        for li, (fi, fo) in enumerate(DIMS):
            last = li == NLAYER - 1
            htil = tpool.tile([128, NBLK, fo], bf16, tag="htil")
            htil2 = tpool.tile([128, NBLK, fo], bf16, tag="htil2")
            # ---- stage A: htil = dinv * (H @ W), htil2 = dinv^2 * (H @ W) ----
            for c in range(NBLK):
                src_ap = x_sb[:, c, :] if li == 0 else h_cur[:, c, :]
                pT = psT.tile([fi, 128], f32, space="PSUM", tag="pT")
                nc.tensor.transpose(out=pT[:], in_=src_ap, identity=ident_t[:])
                hT = work.tile([fi, 128], bf16, tag="hT")
                nc.scalar.activation(
                    out=hT[:], in_=pT[:], func=mybir.ActivationFunctionType.Copy
                )
                pH = psH.tile([128, fo], f32, space="PSUM", tag="pH")
                nc.tensor.matmul(out=pH[:], lhsT=hT[:], rhs=W_ts[li][:], start=True, stop=True)
                nc.scalar.activation(
                    out=htil[:, c, :],
                    in_=pH[:],
                    func=mybir.ActivationFunctionType.Copy,
                    scale=dinv_t[:, c : c + 1],
                )
                nc.scalar.activation(
                    out=htil2[:, c, :],
                    in_=pH[:],
                    func=mybir.ActivationFunctionType.Copy,
                    scale=dinv2_t[:, c : c + 1],
                )
            # ---- stage B/C: allgather htil table ----
            nc.sync.dma_start(out=bounces[li][:], in_=htil[:])
            nc.gpsimd.collective_compute(
                "AllGather",
                mybir.AluOpType.bypass,
                ins=[bounces[li][:]],
                outs=[tables[li][:]],
                replica_groups=[list(range(NC))],
            )
            # ---- stage D: aggregate ----
            h_next = (
                None
                if last
                else hpool.tile([128, NBLK, fo], bf16, tag="h")
            )
            if last:
                out_sb = work.tile([128, NBLK], f32, tag="outsb")

            table_rows = tables[li][:].rearrange("r (c f) -> (r c) f", f=fo)
            msg_tiles = {}

            def ensure_group(g, fo=fo, table_rows=table_rows, msg_tiles=msg_tiles):
                if g in msg_tiles or g * KGRP >= NCH:
                    return
                kk = min(KGRP, NCH - g * KGRP)
                t = msgp.tile([128, KGRP, fo], bf16, tag="msg")
                nc.gpsimd.indirect_dma_start(
                    out=t[:, :kk, :],
                    out_offset=None,
                    in_=table_rows,
                    in_offset=bass.IndirectOffsetOnAxis(
                        ap=idx_t[:, g * KGRP : g * KGRP + kk], axis=0
                    ),
                )
                msg_tiles[g] = t

            for b in range(NBLK):
                acc = psA.tile([128, fo], f32, space="PSUM", tag="acc")
                Mb = int(M_b[b])
                for j in range(Mb):
                    cc = int(cum[b]) + j
                    g, slot = divmod(cc, KGRP)
                    ensure_group(g)
                    ensure_group(g + 1)
                    S = selp.tile([128, 128], bf16, tag="S")
                    nc.vector.tensor_scalar(
                        S[:],
                        iota_t[:],
                        pos_t[:, cc : cc + 1],
                        wgt_t[:, cc : cc + 1],
                        mybir.AluOpType.is_equal,
                        mybir.AluOpType.mult,
                    )
                    nc.tensor.matmul(
                        out=acc[:],
                        lhsT=S[:],
                        rhs=msg_tiles[g][:, slot, :],
                        start=(j == 0),
                        stop=(j == Mb - 1),
                    )
                # out_b = dinv*acc + htil2 + bias ; relu unless last layer
                if last:
                    X = work.tile([128, fo], f32, tag="X")
                else:
                    X = work.tile([128, fo], bf16, tag="X")
                nc.vector.scalar_tensor_tensor(
                    out=X[:],
                    in0=acc[:],
                    scalar=dinv_t[:, b : b + 1],
                    in1=htil2[:, b, :],
                    op0=mybir.AluOpType.mult,
                    op1=mybir.AluOpType.add,
                )
                if last:
                    nc.vector.tensor_tensor(
                        out=out_sb[:, b : b + 1], in0=X[:], in1=B_ts[li][:], op=mybir.AluOpType.add
                    )
                else:
                    X2 = work.tile([128, fo], bf16, tag="X2")
                    nc.vector.tensor_tensor(
                        out=X2[:], in0=X[:], in1=B_ts[li][:], op=mybir.AluOpType.add
                    )
                    nc.scalar.activation(
                        out=h_next[:, b, :], in_=X2[:], func=mybir.ActivationFunctionType.Relu
                    )
            h_cur = h_next
        nc.sync.dma_start(out=out_ext[:], in_=out_sb[:])

    nc.finalize()
    return nc


TRACE = False
LAST_EXEC_NS = None
LAST_TRACE = None


def _prepare(x, edge_index, edge_weight, W1, b1, W2, b2, W3, b3, W4, b4, W5, b5, W6, b6, W7, b7):
    """Host prep + program build. Returns (nc, in_maps)."""
    x = np.asarray(x, dtype=np.float32)
    in_maps, M_b, cum, D = _host_prep(x, np.asarray(edge_index), np.asarray(edge_weight))

    Wmats = [np.asarray(W, dtype=np.float32).astype(BF16) for W in (W1, W2, W3, W4, W5, W6, W7)]
    bvecs = []
    for i, b in enumerate((b1, b2, b3, b4, b5, b6, b7)):
        bt = np.tile(np.asarray(b, dtype=np.float32).reshape(1, -1), (128, 1))
        bvecs.append(bt if i == NLAYER - 1 else bt.astype(BF16))
    iota = np.tile(np.arange(128, dtype=np.float32).reshape(1, 128), (128, 1)).astype(BF16)
    ident = np.eye(128, dtype=np.float32).astype(BF16)
    for m in in_maps:
        for i in range(NLAYER):
            m[f"W{i+1}"] = Wmats[i]
            m[f"b{i+1}"] = bvecs[i]
        m["iota"] = iota
        m["ident"] = ident

    nc = _build_program(M_b, cum, D, None)
    return nc, in_maps


def _postprocess(outs_per_core):
    outs = []
    for k in range(NC):
        pm = outs_per_core[k]  # [128, NBLK], node c*128+p at [p, c]
        outs.append(pm.T.reshape(-1, 1)[:NLOC])
    return np.concatenate(outs, axis=0)


def kernel(x, edge_index, edge_weight, W1, b1, W2, b2, W3, b3, W4, b4, W5, b5, W6, b6, W7, b7):
    from concourse.bass_utils import run_bass_kernel_spmd

    nc, in_maps = _prepare(x, edge_index, edge_weight, W1, b1, W2, b2, W3, b3, W4, b4, W5, b5, W6, b6, W7, b7)
    kw = {}
    if TRACE:
        import tempfile

        kw = dict(trace=True, tmpdir=tempfile.mkdtemp(prefix="gcn_trace_"))
    res = run_bass_kernel_spmd(nc, in_maps, list(range(NC)), **kw)
    global LAST_EXEC_NS, LAST_TRACE
    LAST_EXEC_NS = res.exec_time_ns
    LAST_TRACE = res.instructions_and_trace
    return _postprocess([res.results[k]["out"] for k in range(NC)])


# revision 13
# speedup vs baseline: 1.0369x; 1.0369x over previous
"""GCN (7-layer, PyG GCNConv-style) on 8 Trainium2 NeuronCores.

Strategy (graph-partition data parallel):
- Nodes destination-sharded: core k owns nodes [k*12500, (k+1)*12500).
- Per layer: stage A computes htil = dinv * (H @ W) per local block (bf16),
  AllGathers the compact table, then a single strided DMA expands it into a
  256B-row-pitch padded table for gathering.
- Source nodes are split into 4 segments of 25088 rows so gather indices fit
  int16 (dma_gather requirement). Edge chunks are ordered (segment-major,
  block-minor); gathers use the production dma_gather path (batched, one call
  per GCALL indices).
- Aggregation: per 128-edge chunk, a weighted selection matrix
  S[e,d] = w_e * (dst_pos(e) == d) is built in ONE fused DVE tensor_scalar op
  (is_equal then mult, 4x perf mode on bf16) and accumulated into PSUM by the
  TensorEngine. Per-block accumulators for a whole block-range pass live
  packed inside PSUM bank tiles so the 4 segment phases can accumulate into
  them without spills. Self-loops enter the same accumulation as an
  identity-matrix matmul of htil. Finalize: out = dinv*acc + b (+ReLU).
- Degrees (1 + sum of incoming weights) are computed on device by row
  reduction over a host-permuted zero-padded copy of edge_weight.

Host-side work is index/layout preparation only (sharding, edge sorting,
padding, dtype casts); all floating-point math runs on device.
"""
import sys

sys.path.insert(0, "/opt/trn_rl_repo")

from contextlib import ExitStack

import numpy as np
import ml_dtypes

BF16 = ml_dtypes.bfloat16

NC = 8
N_NODES = 100000
NLOC = N_NODES // NC            # 12500
NBLK = (NLOC + 127) // 128      # 98
NLOCP = NBLK * 128              # 12544
NTAB = NC * NLOCP               # 100352 padded global table rows
DIMS = [(128, 50), (50, 50), (50, 30), (30, 30), (30, 10), (10, 10), (10, 1)]
NLAYER = len(DIMS)

SEG_CORES = 2
NSEG = NC // SEG_CORES          # 4
SEGROWS = SEG_CORES * NLOCP     # 25088 (< 32768, int16-safe)
STRIDE = 128                    # padded table row pitch (256B bf16)
GCALL = 1024                    # indices per dma_gather call (8 chunks)

# block-range passes per layer, sized so PSUM accumulators fit in 3 banks
def _passes(fo):
    per_bank = 512 // fo
    nb = 3 * per_bank  # blocks per pass with 3 PSUM banks
    return [range(s, min(s + nb, NBLK)) for s in range(0, NBLK, nb)]


def _host_prep(x, edge_index, edge_weight):
    row = np.asarray(edge_index[0], dtype=np.int64)
    col = np.asarray(edge_index[1], dtype=np.int64)
    w = np.asarray(edge_weight, dtype=np.float32)

    core_of = col // NLOC
    per_core = []
    cnt_max = np.zeros((NSEG, NBLK), np.int64)
    max_deg = 1
    for k in range(NC):
        m = core_of == k
        r_k = row[m]
        c_k = col[m] - k * NLOC
        w_k = w[m]
        blk = c_k // 128
        pos = c_k % 128
        loc = r_k % NLOC
        src_pad = (r_k // NLOC) * NLOCP + (loc % 128) * NBLK + loc // 128
        seg = src_pad // SEGROWS
        sidx = src_pad % SEGROWS
        order = np.lexsort((blk, seg))  # sort by (seg major, block minor)
        r_k, c_k, w_k, blk, pos, seg, sidx = (
            a[order] for a in (r_k, c_k, w_k, blk, pos, seg, sidx)
        )
        cnt = np.zeros((NSEG, NBLK), np.int64)
        np.add.at(cnt, (seg, blk), 1)
        cnt_max = np.maximum(cnt_max, cnt)
        cdeg = np.bincount(c_k, minlength=NLOCP)
        max_deg = max(max_deg, int(cdeg.max()))
        per_core.append((r_k, c_k, w_k, blk, pos, seg, sidx, cnt))

    M_sb = np.maximum(cnt_max > 0, np.ceil(cnt_max / 128)).astype(np.int64)
    # ensure every block has at least one chunk overall (blocks with zero
    # edges still aggregate fine via the self-loop matmul; M_sb cell can be 0)
    cum = np.zeros(NSEG * NBLK + 1, np.int64)
    cum[1:] = np.cumsum(M_sb.reshape(-1))
    NCH = int(cum[-1])
    D = max_deg

    in_maps = []
    for k in range(NC):
        r_k, c_k, w_k, blk, pos, seg, sidx, cnt = per_core[k]
        n_e = len(r_k)
        cell = seg * NBLK + blk
        first = np.zeros(NSEG * NBLK + 1, np.int64)
        first[1:] = np.cumsum(cnt.reshape(-1))
        rank = np.arange(n_e, dtype=np.int64) - first[cell]
        chunk = cum[cell] + rank // 128
        part = rank % 128

        dst_pos = np.zeros((128, NCH), np.float32)
        w_e = np.zeros((128, NCH), np.float32)
        dst_pos[part, chunk] = pos.astype(np.float32)
        w_e[part, chunk] = w_k

        # int16 gather indices, wrapped (flat j=cc*128+p at [j%16, j//16]),
        # replicated across the 8 Q7 core groups
        flat_idx = np.zeros(NCH * 128, np.int16)
        flat_idx[chunk * 128 + part] = sidx.astype(np.int16)
        idx16 = np.zeros((128, NCH * 8), np.int16)
        wrap = flat_idx.reshape(NCH * 8, 16).T  # [16, NCH*8]
        for g in range(8):
            idx16[16 * g : 16 * g + 16] = wrap

        # padded per-node incoming weights for degree computation
        order2 = np.argsort(c_k, kind="stable")
        c_s = c_k[order2]
        w_s = w_k[order2]
        nfirst = np.zeros(NLOCP + 1, np.int64)
        nfirst[1:] = np.cumsum(np.bincount(c_s, minlength=NLOCP))
        nrank = np.arange(len(c_s), dtype=np.int64) - nfirst[c_s]
        wpad = np.zeros((NLOCP, D), np.float32)
        wpad[c_s, nrank] = w_s
        wpad_dev = wpad.reshape(NBLK, 128, D).transpose(1, 0, 2).copy()

        xk = np.zeros((NLOCP, x.shape[1]), np.float32)
        xk[:NLOC] = x[k * NLOC : (k + 1) * NLOC]
        xk_dev = np.ascontiguousarray(
            xk.reshape(NBLK, 128, x.shape[1]).transpose(1, 0, 2)
        ).astype(BF16)

        in_maps.append(
            {
                "x_p": xk_dev,
                "idx16": idx16,
                "dst_pos": dst_pos,
                "w_e": w_e,
                "wpad": wpad_dev,
            }
        )
    return in_maps, M_sb, cum, D


def _build_program(M_sb, cum, D, weights_shapes):
    from concourse import bass, bacc, mybir, tile

    NCH = int(cum[-1])
    nc = bacc.Bacc("TRN2", target_bir_lowering=False, debug=False, num_devices=NC)

    f32 = mybir.dt.float32
    bf16 = mybir.dt.bfloat16

    x_p = nc.declare_dram_parameter("x_p", [128, NBLK, 128], bf16, isOutput=False)
    idx16_p = nc.declare_dram_parameter("idx16", [128, NCH * 8], mybir.dt.int16, isOutput=False)
    dst_pos = nc.declare_dram_parameter("dst_pos", [128, NCH], f32, isOutput=False)
    w_e = nc.declare_dram_parameter("w_e", [128, NCH], f32, isOutput=False)
    wpad = nc.declare_dram_parameter("wpad", [128, NBLK, D], f32, isOutput=False)
    iota = nc.declare_dram_parameter("iota", [128, 128], bf16, isOutput=False)
    ident = nc.declare_dram_parameter("ident", [128, 128], bf16, isOutput=False)
    Ws, Bs = [], []
    for i, (fi, fo) in enumerate(DIMS):
        Ws.append(nc.declare_dram_parameter(f"W{i+1}", [fi, max(fo, 16)], bf16, isOutput=False))
        bdt = f32 if i == NLAYER - 1 else bf16
        Bs.append(nc.declare_dram_parameter(f"b{i+1}", [128, fo], bdt, isOutput=False))
    out_ext = nc.declare_dram_parameter("out", [128, NBLK], f32, isOutput=True)

    bounces = [nc.dram_tensor(f"bounce{i}", [128, NBLK * max(fo, 16)], bf16) for i, (fi, fo) in enumerate(DIMS)]
    tables_ag = [
        nc.dram_tensor(f"tabag{i}", [NC * 128, NBLK * max(fo, 16)], bf16, addr_space="Shared")
        for i, (fi, fo) in enumerate(DIMS)
    ]
    tables_pad = [
        nc.dram_tensor(f"tabpad{i}", [NTAB, STRIDE], bf16) for i in range(NLAYER)
    ]

    # host-side chunk schedule: per (seg, block) -> [chunk ids]
    chunks_of = {}
    for s in range(NSEG):
        for b in range(NBLK):
            cell = s * NBLK + b
            chunks_of[(s, b)] = list(range(int(cum[cell]), int(cum[cell + 1])))
    last_chunk_of_block = {
        b: max((cc for s in range(NSEG) for cc in chunks_of[(s, b)]), default=None)
        for b in range(NBLK)
    }

    with tile.TileContext(nc) as tc, ExitStack() as ctx:
        const = ctx.enter_context(tc.tile_pool(name="const", bufs=1))
        work = ctx.enter_context(tc.tile_pool(name="work", bufs=2))
        hpool = ctx.enter_context(tc.tile_pool(name="hpool", bufs=2))
        tpool = ctx.enter_context(tc.tile_pool(name="tpool", bufs=2))
        msgp = ctx.enter_context(tc.tile_pool(name="msgp", bufs=4))
        selp = ctx.enter_context(tc.tile_pool(name="selp", bufs=12))
        psT = ctx.enter_context(tc.tile_pool(name="psT", bufs=2, space="PSUM"))
        psH = ctx.enter_context(tc.tile_pool(name="psH", bufs=2, space="PSUM"))
        psB = ctx.enter_context(tc.tile_pool(name="psB", bufs=1, space="PSUM"))

        iota_t = const.tile([128, 128], bf16)
        nc.sync.dma_start(out=iota_t[:], in_=iota[:])
        ident_t = const.tile([128, 128], bf16)
        nc.sync.dma_start(out=ident_t[:], in_=ident[:])
        idx16_t = const.tile([128, NCH * 8], mybir.dt.int16)
        nc.sync.dma_start(out=idx16_t[:], in_=idx16_p[:])
        pos_t = const.tile([128, NCH], f32)
        nc.sync.dma_start(out=pos_t[:], in_=dst_pos[:])
        wgt_t = const.tile([128, NCH], f32)
        nc.sync.dma_start(out=wgt_t[:], in_=w_e[:])
        W_ts, B_ts = [], []
        for i, (fi, fo) in enumerate(DIMS):
            W_t = const.tile([fi, max(fo, 16)], bf16, tag=f"W{i}")
            nc.sync.dma_start(out=W_t[:], in_=Ws[i][:])
            bdt = f32 if i == NLAYER - 1 else bf16
            B_t = const.tile([128, fo], bdt, tag=f"B{i}")
            nc.sync.dma_start(out=B_t[:], in_=Bs[i][:])
            W_ts.append(W_t)
            B_ts.append(B_t)

        # ---- degree -> dinv ----
        with tc.tile_pool(name="wpool", bufs=1) as wpool:
            wpad_t = wpool.tile([128, NBLK, D], f32, tag="wpad")
            nc.sync.dma_start(out=wpad_t[:], in_=wpad[:])
            deg_t = const.tile([128, NBLK], f32)
            for c in range(NBLK):
                nc.vector.tensor_reduce(
                    deg_t[:, c : c + 1],
                    wpad_t[:, c, :],
                    mybir.AxisListType.X,
                    mybir.AluOpType.add,
                )
            sqrt_t = const.tile([128, NBLK], f32)
            nc.scalar.activation(
                out=sqrt_t[:], in_=deg_t[:], func=mybir.ActivationFunctionType.Sqrt, bias=1.0, scale=1.0
            )
            dinv_t = const.tile([128, NBLK], f32)
            nc.vector.reciprocal(out=dinv_t[:], in_=sqrt_t[:])

        h_cur = None
        for li, (fi, fo) in enumerate(DIMS):
            last = li == NLAYER - 1
            fo_ag = max(fo, 16)
            htil = tpool.tile([128, NBLK, fo_ag], bf16, tag="htil")
            # ---- stage A ----
            for c in range(NBLK):
                if li == 0:
                    h_chunk = work.tile([128, fi], bf16, tag="xchunk")
                    nc.sync.dma_start(out=h_chunk[:], in_=x_p[:, c, :])
                    src_ap = h_chunk[:]
                else:
                    src_ap = h_cur[:, c, :]
                pT = psT.tile([fi, 128], bf16, space="PSUM", tag="pT")
                nc.tensor.transpose(out=pT[:], in_=src_ap, identity=ident_t[:])
                hT = work.tile([fi, 128], bf16, tag="hT")
                nc.scalar.activation(out=hT[:], in_=pT[:], func=mybir.ActivationFunctionType.Copy)
                pH = psH.tile([128, fo_ag], f32, space="PSUM", tag="pH")
                nc.tensor.matmul(out=pH[:], lhsT=hT[:], rhs=W_ts[li][:], start=True, stop=True)
                nc.scalar.activation(
                    out=htil[:, c, :],
                    in_=pH[:],
                    func=mybir.ActivationFunctionType.Copy,
                    scale=dinv_t[:, c : c + 1],
                )
            # ---- allgather + pad-expansion ----
            nc.sync.dma_start(out=bounces[li][:], in_=htil[:])
            nc.gpsimd.collective_compute(
                "AllGather",
                mybir.AluOpType.bypass,
                ins=[bounces[li][:]],
                outs=[tables_ag[li][:]],
                replica_groups=[list(range(NC))],
            )
            # split by segment: a single DMA's row count would overflow the
            # 16-bit src_num_elem ISA field at NTAB rows
            tab_rows = tables_ag[li][:].rearrange("r (c f) -> (r c) f", f=fo_ag)
            for s in range(NSEG):
                nc.sync.dma_start(
                    out=tables_pad[li][s * SEGROWS : (s + 1) * SEGROWS, :fo_ag],
                    in_=tab_rows[s * SEGROWS : (s + 1) * SEGROWS, :],
                )

            # ---- aggregation ----
            h_next = None if last else hpool.tile([128, NBLK, fo], bf16, tag="h")
            if last:
                out_sb = work.tile([128, NBLK], f32, tag="outsb")
            per_bank = 512 // fo

            # gather-call schedule for this layer: per (seg, pass) contiguous
            # chunk ranges split into GCALL-index calls
            CHUNKS_PER_CALL = GCALL // 128
            calls = []  # (seg, c0, nch)
            for R in _passes(fo):
                for s in range(NSEG):
                    ccs = [cc for b in R for cc in chunks_of[(s, b)]]
                    if not ccs:
                        continue
                    c0, ce = ccs[0], ccs[-1] + 1
                    assert ccs == list(range(c0, ce))
                    for q in range(c0, ce, CHUNKS_PER_CALL):
                        calls.append((s, q, min(CHUNKS_PER_CALL, ce - q)))
            call_of_chunk = {}
            for i, (s, q, n) in enumerate(calls):
                for cc in range(q, q + n):
                    call_of_chunk[cc] = i
            msg_tiles = {}

            def ensure_call(i, fo=fo, li=li, calls=calls, msg_tiles=msg_tiles):
                if i in msg_tiles or i >= len(calls):
                    return
                s, q, ncc = calls[i]
                t = msgp.tile([128, CHUNKS_PER_CALL, fo], bf16, tag="msg")
                # raw InstDMAGatherAnt emission: elem_size=fo (compact 2*fo-byte
                # descriptors) from 256B-pitch rows — verified exact on HW;
                # the bass dma_gather wrapper over-conservatively requires
                # elem_size_bytes % 256 == 0 (a transpose-mode restriction).
                in_ap = tables_pad[li][s * SEGROWS : (s + 1) * SEGROWS, :fo]
                _in_ap = nc.gpsimd.lower_ap_dma(in_ap, for_custom_bir_dma=True)
                _idxs_ap = nc.gpsimd.lower_ap(idx16_t[:, q * 8 : (q + ncc) * 8])
                _out_ap = nc.gpsimd.lower_ap(t[:, :ncc, :])
                nc.gpsimd.add_instruction(
                    mybir.InstDMAGatherAnt(
                        name=nc.get_next_instruction_name(),
                        ins=[
                            *_in_ap,
                            _idxs_ap,
                            nc.gpsimd.lower_val_access(nc.gpsimd.to_reg(ncc * 128)),
                        ],
                        outs=[_out_ap],
                        transpose=False,
                        num_idxs=ncc * 128,
                        elem_size=fo,
                        stride_bytes_256=1,
                        gen_mode=0,
                        single_packet=True,
                        queue_num=0,
                        sbuf_tokens_per_rank=0,
                        sbuf_free_dim_per_rank=0,
                        sbuf_free_dim_pad_per_rank=0,
                        sbuf_byte_offset=0,
                    )
                )
                msg_tiles[i] = t

            for R in _passes(fo):
                banks = [
                    psB.tile([128, 512], f32, space="PSUM", tag=f"bank{i}", name=f"bank{i}")
                    for i in range(int(np.ceil(len(R) / per_bank)))
                ]

                def acc_ap(b, R=R, banks=banks, per_bank=per_bank, fo=fo):
                    i = b - R[0]
                    return banks[i // per_bank][:, (i % per_bank) * fo : (i % per_bank) * fo + fo]

                # emission schedule; start/stop flags are BANK-granular (a
                # start=True matmul lazily zeroes the whole 2KB zero region)
                sched = [("ident", b, None) for b in R]
                for s in range(NSEG):
                    for b in R:
                        for cc in chunks_of[(s, b)]:
                            sched.append(("chunk", b, cc))
                bank_of = lambda b, R=R, per_bank=per_bank: (b - R[0]) // per_bank
                first_of_bank, last_of_bank = {}, {}
                for i, (_, b, _cc) in enumerate(sched):
                    k = bank_of(b)
                    first_of_bank.setdefault(k, i)
                    last_of_bank[k] = i

                for i, (kind, b, cc) in enumerate(sched):
                    k = bank_of(b)
                    st = first_of_bank[k] == i
                    sp = last_of_bank[k] == i
                    if kind == "ident":
                        nc.tensor.matmul(
                            out=acc_ap(b),
                            lhsT=ident_t[:],
                            rhs=htil[:, b, :fo],
                            start=st,
                            stop=sp,
                        )
                        continue
                    ci = call_of_chunk[cc]
                    ensure_call(ci)
                    ensure_call(ci + 1)
                    sq, q0, _ = calls[ci]
                    slot = cc - q0
                    S = selp.tile([128, 128], bf16, tag="S")
                    nc.vector.tensor_scalar(
                        S[:],
                        iota_t[:],
                        pos_t[:, cc : cc + 1],
                        wgt_t[:, cc : cc + 1],
                        mybir.AluOpType.is_equal,
                        mybir.AluOpType.mult,
                    )
                    nc.tensor.matmul(
                        out=acc_ap(b),
                        lhsT=S[:],
                        rhs=msg_tiles[ci][:, slot, :],
                        start=st,
                        stop=sp,
                    )
                for b in R:
                    if last:
                        nc.vector.scalar_tensor_tensor(
                            out=out_sb[:, b : b + 1],
                            in0=acc_ap(b),
                            scalar=dinv_t[:, b : b + 1],
                            in1=B_ts[li][:],
                            op0=mybir.AluOpType.mult,
                            op1=mybir.AluOpType.add,
                        )
                    else:
                        X2 = work.tile([128, fo], bf16, tag="X2")
                        nc.vector.scalar_tensor_tensor(
                            out=X2[:],
                            in0=acc_ap(b),
                            scalar=dinv_t[:, b : b + 1],
                            in1=B_ts[li][:],
                            op0=mybir.AluOpType.mult,
                            op1=mybir.AluOpType.add,
                        )
                        nc.scalar.activation(
                            out=h_next[:, b, :], in_=X2[:], func=mybir.ActivationFunctionType.Relu
                        )
            h_cur = h_next
        nc.sync.dma_start(out=out_ext[:], in_=out_sb[:])

    nc.finalize()
    return nc


TRACE = False
LAST_EXEC_NS = None
LAST_TRACE = None


def _prepare(x, edge_index, edge_weight, W1, b1, W2, b2, W3, b3, W4, b4, W5, b5, W6, b6, W7, b7):
    x = np.asarray(x, dtype=np.float32)
    in_maps, M_sb, cum, D = _host_prep(x, np.asarray(edge_index), np.asarray(edge_weight))

    Wmats = []
    for i, W in enumerate((W1, W2, W3, W4, W5, W6, W7)):
        Wf = np.asarray(W, dtype=np.float32)
        if Wf.shape[1] < 16:
            Wf = np.concatenate([Wf, np.zeros((Wf.shape[0], 16 - Wf.shape[1]), np.float32)], axis=1)
        Wmats.append(Wf.astype(BF16))
    bvecs = []
    for i, b in enumerate((b1, b2, b3, b4, b5, b6, b7)):
        bt = np.tile(np.asarray(b, dtype=np.float32).reshape(1, -1), (128, 1))
        bvecs.append(bt if i == NLAYER - 1 else bt.astype(BF16))
    iota = np.tile(np.arange(128, dtype=np.float32).reshape(1, 128), (128, 1)).astype(BF16)
    ident = np.eye(128, dtype=np.float32).astype(BF16)
    for m in in_maps:
        for i in range(NLAYER):
            m[f"W{i+1}"] = Wmats[i]
            m[f"b{i+1}"] = bvecs[i]
        m["iota"] = iota
        m["ident"] = ident

    nc = _build_program(M_sb, cum, D, None)
    return nc, in_maps


def _postprocess(outs_per_core):
    outs = []
    for k in range(NC):
        pm = outs_per_core[k]  # [128, NBLK], node c*128+p at [p, c]
        outs.append(pm.T.reshape(-1, 1)[:NLOC])
    return np.concatenate(outs, axis=0)


def kernel(x, edge_index, edge_weight, W1, b1, W2, b2, W3, b3, W4, b4, W5, b5, W6, b6, W7, b7):
    from concourse.bass_utils import run_bass_kernel_spmd

    nc, in_maps = _prepare(x, edge_index, edge_weight, W1, b1, W2, b2, W3, b3, W4, b4, W5, b5, W6, b6, W7, b7)
    kw = {}
    if TRACE:
        import tempfile

        kw = dict(trace=True, tmpdir=tempfile.mkdtemp(prefix="gcn_trace_"))
    res = run_bass_kernel_spmd(nc, in_maps, list(range(NC)), **kw)
    global LAST_EXEC_NS, LAST_TRACE
    LAST_EXEC_NS = res.exec_time_ns
    LAST_TRACE = res.instructions_and_trace
    return _postprocess([res.results[k]["out"] for k in range(NC)])


# revision 15
# speedup vs baseline: 1.1750x; 1.1333x over previous
"""GCN (7-layer, PyG GCNConv-style) on 8 Trainium2 NeuronCores.

Strategy (graph-partition data parallel):
- Nodes destination-sharded: core k owns nodes [k*12500, (k+1)*12500).
- Per layer: stage A computes htil = dinv * (H @ W) per local block (bf16),
  AllGathers the compact table, then a single strided DMA expands it into a
  256B-row-pitch padded table for gathering.
- Source nodes are split into 4 segments of 25088 rows so gather indices fit
  int16 (dma_gather requirement). Edge chunks are ordered (segment-major,
  block-minor); gathers use the production dma_gather path (batched, one call
  per GCALL indices).
- Aggregation: per 128-edge chunk, a weighted selection matrix
  S[e,d] = w_e * (dst_pos(e) == d) is built in ONE fused DVE tensor_scalar op
  (is_equal then mult, 4x perf mode on bf16) and accumulated into PSUM by the
  TensorEngine. Per-block accumulators for a whole block-range pass live
  packed inside PSUM bank tiles so the 4 segment phases can accumulate into
  them without spills. Self-loops enter the same accumulation as an
  identity-matrix matmul of htil. Finalize: out = dinv*acc + b (+ReLU).
- Degrees (1 + sum of incoming weights) are computed on device by row
  reduction over a host-permuted zero-padded copy of edge_weight.

Host-side work is index/layout preparation only (sharding, edge sorting,
padding, dtype casts); all floating-point math runs on device.
"""
import sys

sys.path.insert(0, "/opt/trn_rl_repo")

from contextlib import ExitStack

import numpy as np
import ml_dtypes

BF16 = ml_dtypes.bfloat16

NC = 8
N_NODES = 100000
NLOC = N_NODES // NC            # 12500
NBLK = (NLOC + 127) // 128      # 98
NLOCP = NBLK * 128              # 12544
NTAB = NC * NLOCP               # 100352 padded global table rows
DIMS = [(128, 50), (50, 50), (50, 30), (30, 30), (30, 10), (10, 10), (10, 1)]
NLAYER = len(DIMS)

SEG_CORES = 2
NSEG = NC // SEG_CORES          # 4
SEGROWS = SEG_CORES * NLOCP     # 25088 (< 32768, int16-safe)
STRIDE = 128                    # padded table row pitch (256B bf16)
GCALL = 1024                    # indices per dma_gather call (8 chunks)

# block-range passes per layer, sized so PSUM accumulators fit in 3 banks
def _passes(fo):
    per_bank = 512 // fo
    nb = 3 * per_bank  # blocks per pass with 3 PSUM banks
    return [range(s, min(s + nb, NBLK)) for s in range(0, NBLK, nb)]


def _host_prep(x, edge_index, edge_weight):
    row = np.asarray(edge_index[0], dtype=np.int64)
    col = np.asarray(edge_index[1], dtype=np.int64)
    w = np.asarray(edge_weight, dtype=np.float32)

    core_of = col // NLOC
    per_core = []
    cnt_max = np.zeros((NSEG, NBLK), np.int64)
    max_deg = 1
    for k in range(NC):
        m = core_of == k
        r_k = row[m]
        c_k = col[m] - k * NLOC
        w_k = w[m]
        blk = c_k // 128
        pos = c_k % 128
        loc = r_k % NLOC
        src_pad = (r_k // NLOC) * NLOCP + (loc % 128) * NBLK + loc // 128
        seg = src_pad // SEGROWS
        sidx = src_pad % SEGROWS
        order = np.lexsort((blk, seg))  # sort by (seg major, block minor)
        r_k, c_k, w_k, blk, pos, seg, sidx = (
            a[order] for a in (r_k, c_k, w_k, blk, pos, seg, sidx)
        )
        cnt = np.zeros((NSEG, NBLK), np.int64)
        np.add.at(cnt, (seg, blk), 1)
        cnt_max = np.maximum(cnt_max, cnt)
        cdeg = np.bincount(c_k, minlength=NLOCP)
        max_deg = max(max_deg, int(cdeg.max()))
        per_core.append((r_k, c_k, w_k, blk, pos, seg, sidx, cnt))

    M_sb = np.maximum(cnt_max > 0, np.ceil(cnt_max / 128)).astype(np.int64)
    # ensure every block has at least one chunk overall (blocks with zero
    # edges still aggregate fine via the self-loop matmul; M_sb cell can be 0)
    cum = np.zeros(NSEG * NBLK + 1, np.int64)
    cum[1:] = np.cumsum(M_sb.reshape(-1))
    NCH = int(cum[-1])
    D = max_deg

    in_maps = []
    for k in range(NC):
        r_k, c_k, w_k, blk, pos, seg, sidx, cnt = per_core[k]
        n_e = len(r_k)
        cell = seg * NBLK + blk
        first = np.zeros(NSEG * NBLK + 1, np.int64)
        first[1:] = np.cumsum(cnt.reshape(-1))
        rank = np.arange(n_e, dtype=np.int64) - first[cell]
        chunk = cum[cell] + rank // 128
        part = rank % 128

        dst_pos = np.zeros((128, NCH), np.float32)
        w_e = np.zeros((128, NCH), np.float32)
        dst_pos[part, chunk] = pos.astype(np.float32)
        w_e[part, chunk] = w_k

        # int16 gather indices, wrapped (flat j=cc*128+p at [j%16, j//16]),
        # replicated across the 8 Q7 core groups
        flat_idx = np.zeros(NCH * 128, np.int16)
        flat_idx[chunk * 128 + part] = sidx.astype(np.int16)
        idx16 = np.zeros((128, NCH * 8), np.int16)
        wrap = flat_idx.reshape(NCH * 8, 16).T  # [16, NCH*8]
        for g in range(8):
            idx16[16 * g : 16 * g + 16] = wrap

        # padded per-node incoming weights for degree computation
        order2 = np.argsort(c_k, kind="stable")
        c_s = c_k[order2]
        w_s = w_k[order2]
        nfirst = np.zeros(NLOCP + 1, np.int64)
        nfirst[1:] = np.cumsum(np.bincount(c_s, minlength=NLOCP))
        nrank = np.arange(len(c_s), dtype=np.int64) - nfirst[c_s]
        wpad = np.zeros((NLOCP, D), np.float32)
        wpad[c_s, nrank] = w_s
        wpad_dev = wpad.reshape(NBLK, 128, D).transpose(1, 0, 2).copy()

        xk = np.zeros((NLOCP, x.shape[1]), np.float32)
        xk[:NLOC] = x[k * NLOC : (k + 1) * NLOC]
        xk_dev = np.ascontiguousarray(
            xk.reshape(NBLK, 128, x.shape[1]).transpose(1, 0, 2)
        ).astype(BF16)

        in_maps.append(
            {
                "x_p": xk_dev,
                "idx16": idx16,
                "dst_pos": dst_pos,
                "w_e": w_e,
                "wpad": wpad_dev,
            }
        )
    return in_maps, M_sb, cum, D


def _build_program(M_sb, cum, D, weights_shapes):
    from concourse import bass, bacc, mybir, tile

    NCH = int(cum[-1])
    nc = bacc.Bacc("TRN2", target_bir_lowering=False, debug=False, num_devices=NC)

    f32 = mybir.dt.float32
    bf16 = mybir.dt.bfloat16

    x_p = nc.declare_dram_parameter("x_p", [128, NBLK, 128], bf16, isOutput=False)
    idx16_p = nc.declare_dram_parameter("idx16", [128, NCH * 8], mybir.dt.int16, isOutput=False)
    dst_pos = nc.declare_dram_parameter("dst_pos", [128, NCH], f32, isOutput=False)
    w_e = nc.declare_dram_parameter("w_e", [128, NCH], f32, isOutput=False)
    wpad = nc.declare_dram_parameter("wpad", [128, NBLK, D], f32, isOutput=False)
    iota = nc.declare_dram_parameter("iota", [128, 128], bf16, isOutput=False)
    ident = nc.declare_dram_parameter("ident", [128, 128], bf16, isOutput=False)
    Ws, Bs = [], []
    for i, (fi, fo) in enumerate(DIMS):
        Ws.append(nc.declare_dram_parameter(f"W{i+1}", [fi, max(fo, 16)], bf16, isOutput=False))
        bdt = f32 if i == NLAYER - 1 else bf16
        Bs.append(nc.declare_dram_parameter(f"b{i+1}", [128, fo], bdt, isOutput=False))
    out_ext = nc.declare_dram_parameter("out", [128, NBLK], f32, isOutput=True)

    bounces = [nc.dram_tensor(f"bounce{i}", [128, NBLK * max(fo, 16)], bf16) for i, (fi, fo) in enumerate(DIMS)]
    tables_ag = [
        nc.dram_tensor(f"tabag{i}", [NC * 128, NBLK * max(fo, 16)], bf16, addr_space="Shared")
        for i, (fi, fo) in enumerate(DIMS)
    ]
    tables_pad = [
        nc.dram_tensor(f"tabpad{i}", [NTAB, STRIDE], bf16) for i in range(NLAYER)
    ]

    # host-side chunk schedule: per (seg, block) -> [chunk ids]
    chunks_of = {}
    for s in range(NSEG):
        for b in range(NBLK):
            cell = s * NBLK + b
            chunks_of[(s, b)] = list(range(int(cum[cell]), int(cum[cell + 1])))
    last_chunk_of_block = {
        b: max((cc for s in range(NSEG) for cc in chunks_of[(s, b)]), default=None)
        for b in range(NBLK)
    }

    with tile.TileContext(nc) as tc, ExitStack() as ctx:
        const = ctx.enter_context(tc.tile_pool(name="const", bufs=1))
        work = ctx.enter_context(tc.tile_pool(name="work", bufs=2))
        hpool = ctx.enter_context(tc.tile_pool(name="hpool", bufs=2))
        tpool = ctx.enter_context(tc.tile_pool(name="tpool", bufs=2))
        msgp = ctx.enter_context(tc.tile_pool(name="msgp", bufs=4))
        selp = ctx.enter_context(tc.tile_pool(name="selp", bufs=12))
        psT = ctx.enter_context(tc.tile_pool(name="psT", bufs=2, space="PSUM"))
        psH = ctx.enter_context(tc.tile_pool(name="psH", bufs=2, space="PSUM"))
        psB = ctx.enter_context(tc.tile_pool(name="psB", bufs=1, space="PSUM"))

        iota_t = const.tile([128, 128], bf16)
        nc.sync.dma_start(out=iota_t[:], in_=iota[:])
        ident_t = const.tile([128, 128], bf16)
        nc.sync.dma_start(out=ident_t[:], in_=ident[:])
        idx16_t = const.tile([128, NCH * 8], mybir.dt.int16)
        nc.sync.dma_start(out=idx16_t[:], in_=idx16_p[:])
        pos_t = const.tile([128, NCH], f32)
        nc.sync.dma_start(out=pos_t[:], in_=dst_pos[:])
        wgt_t = const.tile([128, NCH], f32)
        nc.sync.dma_start(out=wgt_t[:], in_=w_e[:])
        W_ts, B_ts = [], []
        for i, (fi, fo) in enumerate(DIMS):
            W_t = const.tile([fi, max(fo, 16)], bf16, tag=f"W{i}")
            nc.sync.dma_start(out=W_t[:], in_=Ws[i][:])
            bdt = f32 if i == NLAYER - 1 else bf16
            B_t = const.tile([128, fo], bdt, tag=f"B{i}")
            nc.sync.dma_start(out=B_t[:], in_=Bs[i][:])
            W_ts.append(W_t)
            B_ts.append(B_t)

        # ---- degree -> dinv ----
        with tc.tile_pool(name="wpool", bufs=1) as wpool:
            wpad_t = wpool.tile([128, NBLK, D], f32, tag="wpad")
            nc.sync.dma_start(out=wpad_t[:], in_=wpad[:])
            deg_t = const.tile([128, NBLK], f32)
            for c in range(NBLK):
                nc.vector.tensor_reduce(
                    deg_t[:, c : c + 1],
                    wpad_t[:, c, :],
                    mybir.AxisListType.X,
                    mybir.AluOpType.add,
                )
            sqrt_t = const.tile([128, NBLK], f32)
            nc.scalar.activation(
                out=sqrt_t[:], in_=deg_t[:], func=mybir.ActivationFunctionType.Sqrt, bias=1.0, scale=1.0
            )
            dinv_t = const.tile([128, NBLK], f32)
            nc.vector.reciprocal(out=dinv_t[:], in_=sqrt_t[:])

        h_cur = None
        for li, (fi, fo) in enumerate(DIMS):
            last = li == NLAYER - 1
            fo_ag = max(fo, 16)
            htil = tpool.tile([128, NBLK, fo_ag], bf16, tag="htil")
            # ---- stage A ----
            for c in range(NBLK):
                if li == 0:
                    h_chunk = work.tile([128, fi], bf16, tag="xchunk")
                    nc.sync.dma_start(out=h_chunk[:], in_=x_p[:, c, :])
                    src_ap = h_chunk[:]
                else:
                    src_ap = h_cur[:, c, :]
                pT = psT.tile([fi, 128], bf16, space="PSUM", tag="pT")
                nc.tensor.transpose(out=pT[:], in_=src_ap, identity=ident_t[:])
                hT = work.tile([fi, 128], bf16, tag="hT")
                nc.scalar.activation(out=hT[:], in_=pT[:], func=mybir.ActivationFunctionType.Copy)
                pH = psH.tile([128, fo_ag], f32, space="PSUM", tag="pH")
                nc.tensor.matmul(out=pH[:], lhsT=hT[:], rhs=W_ts[li][:], start=True, stop=True)
                nc.scalar.activation(
                    out=htil[:, c, :],
                    in_=pH[:],
                    func=mybir.ActivationFunctionType.Copy,
                    scale=dinv_t[:, c : c + 1],
                )
            # ---- allgather + pad-expansion ----
            nc.sync.dma_start(out=bounces[li][:], in_=htil[:])
            nc.gpsimd.collective_compute(
                "AllGather",
                mybir.AluOpType.bypass,
                ins=[bounces[li][:]],
                outs=[tables_ag[li][:]],
                replica_groups=[list(range(NC))],
            )
            # split by segment: a single DMA's row count would overflow the
            # 16-bit src_num_elem ISA field at NTAB rows
            tab_rows = tables_ag[li][:].rearrange("r (c f) -> (r c) f", f=fo_ag)
            for s in range(NSEG):
                nc.sync.dma_start(
                    out=tables_pad[li][s * SEGROWS : (s + 1) * SEGROWS, :fo_ag],
                    in_=tab_rows[s * SEGROWS : (s + 1) * SEGROWS, :],
                )

            # ---- aggregation ----
            h_next = None if last else hpool.tile([128, NBLK, fo], bf16, tag="h")
            if last:
                out_sb = work.tile([128, NBLK], f32, tag="outsb")
            per_bank = 512 // fo

            # gather-call schedule for this layer: per (seg, pass) contiguous
            # chunk ranges split into GCALL-index calls
            CHUNKS_PER_CALL = GCALL // 128
            calls = []  # (seg, c0, nch)
            for R in _passes(fo):
                for s in range(NSEG):
                    ccs = [cc for b in R for cc in chunks_of[(s, b)]]
                    if not ccs:
                        continue
                    c0, ce = ccs[0], ccs[-1] + 1
                    assert ccs == list(range(c0, ce))
                    for q in range(c0, ce, CHUNKS_PER_CALL):
                        calls.append((s, q, min(CHUNKS_PER_CALL, ce - q)))
            call_of_chunk = {}
            for i, (s, q, n) in enumerate(calls):
                for cc in range(q, q + n):
                    call_of_chunk[cc] = i
            msg_tiles = {}

            def ensure_call(i, fo=fo, li=li, calls=calls, msg_tiles=msg_tiles):
                if i in msg_tiles or i >= len(calls):
                    return
                s, q, ncc = calls[i]
                t = msgp.tile([128, CHUNKS_PER_CALL, fo], bf16, tag="msg")
                # raw InstDMAGatherAnt emission: elem_size=fo (compact 2*fo-byte
                # descriptors) from 256B-pitch rows — verified exact on HW;
                # the bass dma_gather wrapper over-conservatively requires
                # elem_size_bytes % 256 == 0 (a transpose-mode restriction).
                in_ap = tables_pad[li][s * SEGROWS : (s + 1) * SEGROWS, :fo]
                _in_ap = nc.gpsimd.lower_ap_dma(in_ap, for_custom_bir_dma=True)
                _idxs_ap = nc.gpsimd.lower_ap(idx16_t[:, q * 8 : (q + ncc) * 8])
                _out_ap = nc.gpsimd.lower_ap(t[:, :ncc, :])
                nc.gpsimd.add_instruction(
                    mybir.InstDMAGatherAnt(
                        name=nc.get_next_instruction_name(),
                        ins=[
                            *_in_ap,
                            _idxs_ap,
                            nc.gpsimd.lower_val_access(nc.gpsimd.to_reg(ncc * 128)),
                        ],
                        outs=[_out_ap],
                        transpose=False,
                        num_idxs=ncc * 128,
                        elem_size=fo,
                        stride_bytes_256=1,
                        gen_mode=0,
                        single_packet=True,
                        queue_num=0,
                        sbuf_tokens_per_rank=0,
                        sbuf_free_dim_per_rank=0,
                        sbuf_free_dim_pad_per_rank=0,
                        sbuf_byte_offset=0,
                    )
                )
                msg_tiles[i] = t

            for R in _passes(fo):
                banks = [
                    psB.tile([128, 512], f32, space="PSUM", tag=f"bank{i}", name=f"bank{i}")
                    for i in range(int(np.ceil(len(R) / per_bank)))
                ]

                def acc_ap(b, R=R, banks=banks, per_bank=per_bank, fo=fo):
                    i = b - R[0]
                    return banks[i // per_bank][:, (i % per_bank) * fo : (i % per_bank) * fo + fo]

                # emission schedule; start/stop flags are BANK-granular (a
                # start=True matmul lazily zeroes the whole 2KB zero region)
                sched = [("ident", b, None) for b in R]
                for s in range(NSEG):
                    for b in R:
                        for cc in chunks_of[(s, b)]:
                            sched.append(("chunk", b, cc))
                bank_of = lambda b, R=R, per_bank=per_bank: (b - R[0]) // per_bank
                first_of_bank, last_of_bank = {}, {}
                for i, (_, b, _cc) in enumerate(sched):
                    k = bank_of(b)
                    first_of_bank.setdefault(k, i)
                    last_of_bank[k] = i

                for i, (kind, b, cc) in enumerate(sched):
                    k = bank_of(b)
                    st = first_of_bank[k] == i
                    sp = last_of_bank[k] == i
                    if kind == "ident":
                        nc.tensor.matmul(
                            out=acc_ap(b),
                            lhsT=ident_t[:],
                            rhs=htil[:, b, :fo],
                            start=st,
                            stop=sp,
                        )
                        continue
                    ci = call_of_chunk[cc]
                    ensure_call(ci)
                    ensure_call(ci + 1)
                    ensure_call(ci + 2)
                    sq, q0, _ = calls[ci]
                    slot = cc - q0
                    S = selp.tile([128, 128], bf16, tag="S")
                    nc.vector.tensor_scalar(
                        S[:],
                        iota_t[:],
                        pos_t[:, cc : cc + 1],
                        wgt_t[:, cc : cc + 1],
                        mybir.AluOpType.is_equal,
                        mybir.AluOpType.mult,
                    )
                    nc.tensor.matmul(
                        out=acc_ap(b),
                        lhsT=S[:],
                        rhs=msg_tiles[ci][:, slot, :],
                        start=st,
                        stop=sp,
                    )
                for b in R:
                    if last:
                        nc.vector.scalar_tensor_tensor(
                            out=out_sb[:, b : b + 1],
                            in0=acc_ap(b),
                            scalar=dinv_t[:, b : b + 1],
                            in1=B_ts[li][:],
                            op0=mybir.AluOpType.mult,
                            op1=mybir.AluOpType.add,
                        )
                    else:
                        X2 = work.tile([128, fo], bf16, tag="X2")
                        nc.vector.scalar_tensor_tensor(
                            out=X2[:],
                            in0=acc_ap(b),
                            scalar=dinv_t[:, b : b + 1],
                            in1=B_ts[li][:],
                            op0=mybir.AluOpType.mult,
                            op1=mybir.AluOpType.add,
                        )
                        nc.scalar.activation(
                            out=h_next[:, b, :], in_=X2[:], func=mybir.ActivationFunctionType.Relu
                        )
            h_cur = h_next
        nc.sync.dma_start(out=out_ext[:], in_=out_sb[:])

    nc.finalize()
    return nc


TRACE = False
LAST_EXEC_NS = None
LAST_TRACE = None


def _prepare(x, edge_index, edge_weight, W1, b1, W2, b2, W3, b3, W4, b4, W5, b5, W6, b6, W7, b7):
    x = np.asarray(x, dtype=np.float32)
    in_maps, M_sb, cum, D = _host_prep(x, np.asarray(edge_index), np.asarray(edge_weight))

    Wmats = []
    for i, W in enumerate((W1, W2, W3, W4, W5, W6, W7)):
        Wf = np.asarray(W, dtype=np.float32)
        if Wf.shape[1] < 16:
            Wf = np.concatenate([Wf, np.zeros((Wf.shape[0], 16 - Wf.shape[1]), np.float32)], axis=1)
        Wmats.append(Wf.astype(BF16))
    bvecs = []
    for i, b in enumerate((b1, b2, b3, b4, b5, b6, b7)):
        bt = np.tile(np.asarray(b, dtype=np.float32).reshape(1, -1), (128, 1))
        bvecs.append(bt if i == NLAYER - 1 else bt.astype(BF16))
    iota = np.tile(np.arange(128, dtype=np.float32).reshape(1, 128), (128, 1)).astype(BF16)
    ident = np.eye(128, dtype=np.float32).astype(BF16)
    for m in in_maps:
        for i in range(NLAYER):
            m[f"W{i+1}"] = Wmats[i]
            m[f"b{i+1}"] = bvecs[i]
        m["iota"] = iota
        m["ident"] = ident

    nc = _build_program(M_sb, cum, D, None)
    return nc, in_maps


def _postprocess(outs_per_core):
    outs = []
    for k in range(NC):
        pm = outs_per_core[k]  # [128, NBLK], node c*128+p at [p, c]
        outs.append(pm.T.reshape(-1, 1)[:NLOC])
    return np.concatenate(outs, axis=0)


def kernel(x, edge_index, edge_weight, W1, b1, W2, b2, W3, b3, W4, b4, W5, b5, W6, b6, W7, b7):
    from concourse.bass_utils import run_bass_kernel_spmd

    nc, in_maps = _prepare(x, edge_index, edge_weight, W1, b1, W2, b2, W3, b3, W4, b4, W5, b5, W6, b6, W7, b7)
    kw = {}
    if TRACE:
        import tempfile

        kw = dict(trace=True, tmpdir=tempfile.mkdtemp(prefix="gcn_trace_"))
    res = run_bass_kernel_spmd(nc, in_maps, list(range(NC)), **kw)
    global LAST_EXEC_NS, LAST_TRACE
    LAST_EXEC_NS = res.exec_time_ns
    LAST_TRACE = res.instructions_and_trace
    return _postprocess([res.results[k]["out"] for k in range(NC)])
